# revision 2
# baseline (speedup 1.0000x reference)
"""JSD loss kernel for Trainium2 (8 NeuronCores, row-sharded SPMD), v2.

loss[r] = beta*KL(P||M) + (1-beta)*KL(Q||M), beta=0.5, M=(P+Q)/2
        = sum_v [ p'*lp + q'*lq - m*log(m) ]   (p'=p/2, q'=q/2, m=p'+q')

Engine assignment per 128x2048 chunk (per core: 256 rows x 32000 vocab):
  ACT : pq = Exp(lplq - ln2)  (one instr over 2C cols; fp32r out)
  PE  : m  = p' + q'  via fp32r identity matmuls (1 cycle/row vs 4 for
        plain fp32; fp32r consumers need fp32r-rounded producers, hence
        the fp32r pq tile and the rounded identity)
  ACT : logm = Ln(m)  (from PSUM)
  DVE : STT accum  Sum(lplq * pq) over 2C  (= Sum p'lp + Sum q'lq)
  DVE : STT accum  Sum(m * logm)  over C
DVE is the bottleneck (~207us busy): it is the only engine that can do
product+reduce (the Pool/GPSIMD core-v3 ISA has no TensorScalarPtr and no
PSUM access), and it runs 1 elem/lane/cycle at 0.96 GHz for 3 column-visits
per vocab element. DMA 182us (65.5MB @ 360GB/s), ACT ~175us (3 visits at
1.2 GHz; a single act-table load thanks to the combined exp+ln table), PE
~105us. Cost model: ~227us vs ~289us for the fp32-matmul baseline.
"""

import sys
from contextlib import ExitStack

import numpy as np

sys.path.insert(0, "/opt/trn_rl_repo")

N, V = 2048, 32000
NCORES = 8
R = N // NCORES  # rows per core = 256
P = 128  # partitions
NBLK = R // P  # row blocks per core = 2
# 32000 cols per row-block, organized as DMA/Exp groups of compute
# sub-chunks. Each group gets ONE lp DMA + ONE lq DMA + ONE Exp instruction
# (over 2*G cols); each sub-chunk gets its own PE-add/Ln/STT set (PSUM bank
# granularity caps sub-chunks at 2048 f32 = 4 banks, 2-deep ring). Grouping
# halves the per-instruction overhead on ACT (the near-critical engine) and
# the HWDGE/SP DMA dispatch count.  Tapered only at the program edges: small
# head groups on block 0 shorten the pipeline fill, small tail groups on
# block 1 shorten the serial drain after the last input DMA. Sub-chunk sizes
# are multiples of 512 (PSUM-bank-aligned matmul slabs) except the 256 tail
# (fp32r needs >=256-wide matmul output for 1 cycle/row).
GROUPS_BY_BLOCK = [
    [[256], [512], [1280], *([[2048]] * 14), [1280]],
    [*([[2048]] * 15), [1024], [256]],
]
assert all(sum(sum(g) for g in blk) == V for blk in GROUPS_BY_BLOCK)
LN2 = 0.6931471805599453
# Number of PE warm-keeping dummy matmuls per sub-chunk (0 = disabled).
WARM_PE = 0

_CACHE = {}


def _combined_act_tables(real_fn, mybir):
    """Route both Exp and Ln to the one act-func table that contains both.

    Bass's insert_act_table_loads picks, per activation function, the first
    act_info.json table containing it: Exp -> exp_and_others, Ln ->
    natural_log, which ping-pongs a 1283ns table load per chunk (41us of
    Activation-engine time).  Hiding every other table from the placement
    pass makes first-match land on natural_log_exp_and_others for both, so
    exactly one load is emitted.  Table *indices* (what walrus consumes as
    act_func_set_id) are preserved because entries are kept in order.
    """

    def wrapper(arch):
        tables = real_fn(arch)
        Exp = mybir.ActivationFunctionType.Exp
        Ln = mybir.ActivationFunctionType.Ln
        combined = next(
            (name for name, fs in tables.items() if Exp in fs and Ln in fs), None
        )
        if combined is None:
            return tables
        return {
            name: (fs if name == combined else set()) for name, fs in tables.items()
        }

    return wrapper


def _build_program():
    import concourse.bacc as bacc
    import concourse.tile as tile
    from concourse import mybir

    nc = bacc.Bacc(
        "TRN2",
        target_bir_lowering=False,
        debug=False,
        enable_asserts=False,
        num_devices=1,
    )
    lp_d = nc.dram_tensor("log_p", [R, V], mybir.dt.float32, kind="ExternalInput")
    lq_d = nc.dram_tensor("log_q", [R, V], mybir.dt.float32, kind="ExternalInput")
    id_d = nc.dram_tensor("ident", [P, P], mybir.dt.float32, kind="ExternalInput")
    out_d = nc.dram_tensor("loss", [R, 1], mybir.dt.float32, kind="ExternalOutput")

    lp = lp_d.ap()
    lq = lq_d.ap()
    out = out_d.ap()

    fp32 = mybir.dt.float32
    fp32r = mybir.dt.float32r
    bf16 = mybir.dt.bfloat16
    Exp = mybir.ActivationFunctionType.Exp
    Ln = mybir.ActivationFunctionType.Ln
    mult = mybir.AluOpType.mult

    with tile.TileContext(nc) as tc, ExitStack() as ctx:
        const = ctx.enter_context(tc.tile_pool(name="const", bufs=1))
        loads = ctx.enter_context(tc.tile_pool(name="loads", bufs=6))
        acts = ctx.enter_context(tc.tile_pool(name="acts", bufs=4))
        logms = ctx.enter_context(tc.tile_pool(name="logms", bufs=3))
        jnk_v = ctx.enter_context(tc.tile_pool(name="jnkv", bufs=1))
        parts = ctx.enter_context(tc.tile_pool(name="parts", bufs=2))
        outs = ctx.enter_context(tc.tile_pool(name="outs", bufs=2))
        psum = ctx.enter_context(tc.tile_pool(name="psum", bufs=2, space="PSUM"))

        ident_sb = const.tile([P, P], fp32)
        nc.sync.dma_start(out=ident_sb[:], in_=id_d.ap())
        neg_ln2 = const.tile([P, 1], fp32)
        nc.vector.memset(neg_ln2[:], -LN2)
        # fp32r consumers require producers that round to fp32r; round the
        # identity once through the Activation copy path (Copy is in every
        # act table, so this costs no table load).
        ident_r = const.tile([P, P], fp32r)
        nc.scalar.activation(
            out=ident_r[:], in_=ident_sb[:], func=mybir.ActivationFunctionType.Copy
        )

        # junk outputs for the three STTs (values unused; accum_out is the
        # real output). One tile per engine so there are no cross-engine
        # WAW deps; same-engine WAW is serial anyway.
        junk_dve = jnk_v.tile([P, 3 * 2048], bf16)

        warm_src = None
        if WARM_PE:
            warm_src = const.tile([P, 512], fp32)
            nc.vector.memset(warm_src[:], 0.0)

        def m_tile_for(C):
            return psum.tile([P, 2048], fp32, tag="m", name="m")

        for b in range(NBLK):
            groups = GROUPS_BY_BLOCK[b]
            nch = sum(len(g) for g in groups)
            r0 = b * P
            ab1_parts = parts.tile([P, nch], fp32, tag="ab1")
            c_parts = parts.tile([P, nch], fp32, tag="cp")
            i = 0  # sub-chunk index within the block
            c0 = 0  # column offset within the block
            for group in groups:
                G = sum(group)
                # tile layout: [ lp cols 0:G | lq cols G:2G ]
                lplq = loads.tile([P, 2 * 2048], fp32, tag="lplq")
                pq = acts.tile([P, 2 * 2048], fp32r, tag="pq")
                nc.sync.dma_start(out=lplq[:, 0:G], in_=lp[r0 : r0 + P, c0 : c0 + G])
                nc.sync.dma_start(
                    out=lplq[:, G : 2 * G], in_=lq[r0 : r0 + P, c0 : c0 + G]
                )
                # p' = exp(lp - ln2) = p/2 ; q' = q/2  (single instr, 2G cols)
                nc.scalar.activation(
                    out=pq[:, 0 : 2 * G],
                    in_=lplq[:, 0 : 2 * G],
                    func=Exp,
                    bias=neg_ln2[:],
                )
                o = 0  # sub-chunk offset within the group
                for C in group:
                    # m = p' + q'  (fp32r identity matmuls: 1 cycle/row)
                    m_ps = m_tile_for(C)
                    if WARM_PE:
                        # Dummy matmuls into the fresh PSUM tile keep the PE
                        # continuously busy so it holds its high p-state: a
                        # cold PE runs matmuls 2-4x slower, which sits on the
                        # Exp -> PE -> Ln critical path. The first real
                        # matmul below has start=True, which resets PSUM, so
                        # whatever the dummies wrote is discarded.
                        for _ in range(WARM_PE):
                            nc.tensor.matmul(
                                out=m_ps[:, 0:512],
                                lhsT=ident_sb[:].bitcast(fp32r),
                                rhs=warm_src[:].bitcast(fp32r),
                                start=True,
                                stop=True,
                                skip_group_check=True,
                            )
                    for j0 in range(0, C, 512):
                        w = min(512, C - j0)
                        nc.tensor.matmul(
                            out=m_ps[:, j0 : j0 + w],
                            lhsT=ident_r[:],
                            rhs=pq[:, o + j0 : o + j0 + w],
                            start=True,
                            stop=False,
                        )
                        nc.tensor.matmul(
                            out=m_ps[:, j0 : j0 + w],
                            lhsT=ident_r[:],
                            rhs=pq[:, G + o + j0 : G + o + j0 + w],
                            start=False,
                            stop=True,
                        )
                    logm = logms.tile([P, 2048], fp32, tag="logm")
                    nc.scalar.activation(out=logm[:, 0:C], in_=m_ps[:, 0:C], func=Ln)

                    # DVE: sum(lp*p' + lq*q') in one pass over the
                    # concatenated [lp|lq] x [p'|q'] tiles. (Pool's core-v3
                    # ISA has no TensorScalarPtr and no PSUM access, so all
                    # product+reduce work lives on DVE.)
                    nc.vector.scalar_tensor_tensor(
                        out=junk_dve[:, 0 : 2 * C],
                        in0=lplq[:, o : o + 2 * C],
                        scalar=1.0,
                        in1=pq[:, o : o + 2 * C].bitcast(fp32),
                        op0=mult,
                        op1=mult,
                        accum_out=ab1_parts[:, i : i + 1],
                    )
                    # DVE: sum(m * logm) — issued first so the PSUM buffer
                    # is freed as early as possible (the 2-deep PSUM ring
                    # gates how far ahead the PE/Ln pipeline can run).
                    nc.vector.scalar_tensor_tensor(
                        out=junk_dve[:, 2 * 2048 : 2 * 2048 + C],
                        in0=logm[:, 0:C],
                        scalar=1.0,
                        in1=m_ps[:, 0:C],
                        op0=mult,
                        op1=mult,
                        accum_out=c_parts[:, i : i + 1],
                    )
                    o += C
                    i += 1
                c0 += G
            d_parts = parts.tile([P, nch], fp32, tag="dp")
            nc.vector.tensor_sub(d_parts[:], ab1_parts[:], c_parts[:])
            loss_b = outs.tile([P, 1], fp32)
            nc.vector.reduce_sum(
                out=loss_b[:], in_=d_parts[:], axis=mybir.AxisListType.X
            )
            nc.sync.dma_start(out=out[r0 : r0 + P, :], in_=loss_b[:])

    real_fn = bacc.get_activation_tables
    bacc.get_activation_tables = _combined_act_tables(real_fn, mybir)
    try:
        nc.compile()
    finally:
        bacc.get_activation_tables = real_fn
    return nc


def _get_program():
    if "nc" not in _CACHE:
        _CACHE["nc"] = _build_program()
    return _CACHE["nc"]


def kernel(log_q: np.ndarray, log_p: np.ndarray, _trace: bool = False):
    from concourse.bass_utils import run_bass_kernel_spmd

    log_q = np.ascontiguousarray(np.asarray(log_q, dtype=np.float32))
    log_p = np.ascontiguousarray(np.asarray(log_p, dtype=np.float32))
    assert log_q.shape == (N, V) and log_p.shape == (N, V)

    nc = _get_program()
    ident = np.eye(P, dtype=np.float32)
    in_maps = []
    for c in range(NCORES):
        sl = slice(c * R, (c + 1) * R)
        in_maps.append({"log_p": log_p[sl], "log_q": log_q[sl], "ident": ident})
    res = run_bass_kernel_spmd(
        nc, in_maps, core_ids=list(range(NCORES)), trace=_trace
    )
    _CACHE["last_results"] = res
    outs = [res.results[c]["loss"].reshape(R) for c in range(NCORES)]
    return np.concatenate(outs, axis=0).astype(np.float32)
